# revision 110
# baseline (speedup 1.0000x reference)
"""Multi-head attention layer (L=2048, B=2, D=1024, H=16) on 8 Trainium2 cores.

Sharding: batch*heads across cores — core c handles batch c//4, heads
4*(c%4)..4*(c%4)+4.  Tensor-parallel W_in column slice (per-head) and W_out
row slice; per-core partial outputs are summed on the host (2 groups of 4).

v33: deadline-scheduled fill queue.  The kernel is a sequence of ten
single-head attention blocks (S = K^T Q per key-chunk mc, exp on ACT,
AV accumulate, softmax-normalize).  All projection / output work is a
queue of fill closures with (block, slot) deadlines; each mc slot first
emits every due fill, then pulls optional fills while emitted PE time
trails emitted ACT time, keeping both engines saturated.  Block tails
(last AVs, normalize, z transposes) are deferred into the next block's
early slots so the PE crosses block boundaries without draining.
z is accumulated token-major (65-wide AV with a ones column for row
sums), normalized into a persistent [q, chunk, pair, j] buffer, and
moved to dim-major via DMA-engine XBAR transposes (off PE/DVE/ACT).
All matmul operands f16; f16 output DMA; b_out added on host.
"""

import sys

for _p in ("/opt/trn_rl_repo",):
    if _p not in sys.path:
        sys.path.append(_p)

import numpy as np

L, B, D, H = 2048, 2, 1024, 16
HD = 64
NCORES = 8
HPC = 4              # heads per core
J = HPC * HD         # 256 per-core head-dim slice
KC = D // 128        # 8 contraction chunks
P = 128

_COMPILED = None


def _build():
    import concourse.bacc as bacc
    import concourse.mybir as mybir
    import concourse.tile as tile
    from contextlib import ExitStack

    f32 = mybir.dt.float32
    f16 = mybir.dt.float16
    Exp = mybir.ActivationFunctionType.Exp
    Mult = mybir.AluOpType.mult
    Add = mybir.AluOpType.add

    nc = bacc.Bacc("TRN2", target_bir_lowering=False, debug=False)

    xT_d = nc.dram_tensor("xT", (D, L), f16, kind="ExternalInput")
    wqk_d = nc.dram_tensor("wqkT", (2 * J, KC * P), f16, kind="ExternalInput")
    wv_d = nc.dram_tensor("wvT", (D, J), f16, kind="ExternalInput")
    wo_d = nc.dram_tensor("woT", (J, D), f16, kind="ExternalInput")
    ident_d = nc.dram_tensor("ident", (P, P), f16, kind="ExternalInput")
    out_d = nc.dram_tensor("out_p", (L, D), f16, kind="ExternalOutput")

    with tile.TileContext(nc) as tc, ExitStack() as ctx:
        pers = ctx.enter_context(tc.tile_pool(name="pers", bufs=1))
        psum = ctx.enter_context(tc.tile_pool(name="psum", bufs=2, space="PSUM"))
        att = ctx.enter_context(tc.tile_pool(name="att", bufs=3))

        qk_sb = pers.tile([P, 4, L], f16)           # jc 0,1: q pairs; 2,3: k pairs
        v_sb = pers.tile([P, 16, HPC, P], f16)      # v cols 0:64, ones col 64
        zn_sb = pers.tile([P, 2, L], f16)           # dim-major normalized z per pair
        ztm_sb = pers.tile([P, 16, 2, P], f16)      # token-major z [q, chunk, pair, j]
        wo_sb = pers.tile([P, 2, D], f16)
        xT_sb = pers.tile([P, KC, L], f16)
        wqk_sb = pers.tile([P, 4, KC, P], f16)
        wv_sb = pers.tile([P, KC, J], f16)
        id_sb = pers.tile([P, P], f16)

        # PE warm-up: a chain of dummy matmuls keeps the PE busy through the
        # initial DMA window so the pstate ramp elapses before real work
        # (an idle gap resets pe_busy_start).
        warm = pers.tile([P, 512], f16)
        nc.vector.memset(warm[:], 0.0)
        wp = psum.tile([P, 512], f32, tag="z", bufs=1, name="warm")
        for _ in range(6):
            nc.tensor.matmul(wp[0:1, :], warm[:, 0:1], warm[:], start=True,
                             stop=True)

        out_ap = out_d.ap().rearrange("(t p) o -> p t o", p=P)
        xT_ap = xT_d.ap().rearrange("(kc p) m -> p kc m", p=P)
        wqk_ap = wqk_d.ap().rearrange("(jc p) f -> p jc f", p=P)
        wv_ap = wv_d.ap().rearrange("(kc p) j -> p kc j", p=P)
        wo_ap = wo_d.ap().rearrange("(dc p) o -> p dc o", p=P)

        # stripe DMAs ordered for earliest prologue start
        nc.sync.dma_start(wqk_sb[:, 0].rearrange("p kc c -> p (kc c)"), wqk_ap[:, 0])
        nc.scalar.dma_start(xT_sb[:, 0:4, 0:512], xT_ap[:, 0:4, 0:512])
        nc.sync.dma_start(wqk_sb[:, 2].rearrange("p kc c -> p (kc c)"), wqk_ap[:, 2])
        nc.scalar.dma_start(xT_sb[:, 4:8, 0:512], xT_ap[:, 4:8, 0:512])
        nc.sync.dma_start(wv_sb[:], wv_ap[:])
        nc.scalar.dma_start(xT_sb[:, 0:4, 512:1024], xT_ap[:, 0:4, 512:1024])
        nc.sync.dma_start(xT_sb[:, 4:8, 512:1024], xT_ap[:, 4:8, 512:1024])
        nc.scalar.dma_start(wqk_sb[:, 1].rearrange("p kc c -> p (kc c)"), wqk_ap[:, 1])
        nc.sync.dma_start(wqk_sb[:, 3].rearrange("p kc c -> p (kc c)"), wqk_ap[:, 3])
        nc.scalar.dma_start(xT_sb[:, 0:4, 1024:1536], xT_ap[:, 0:4, 1024:1536])
        nc.sync.dma_start(xT_sb[:, 4:8, 1024:1536], xT_ap[:, 4:8, 1024:1536])
        nc.scalar.dma_start(xT_sb[:, 0:4, 1536:2048], xT_ap[:, 0:4, 1536:2048])
        nc.sync.dma_start(xT_sb[:, 4:8, 1536:2048], xT_ap[:, 4:8, 1536:2048])
        nc.scalar.dma_start(wo_sb[:], wo_ap[:])

        # ---- emitted-work clocks (ns) for greedy fill balancing ----
        clock = {"pe": 0.0, "act": 0.0}
        CYC = 0.4167

        # ---- fill primitives ----
        def qk_chunk(jc, c0, w=128):
            pt = psum.tile([P, w], f32, tag="S", bufs=3, name=f"qkp_{jc}_{c0}")
            for kc in range(KC):
                nc.tensor.matmul(
                    pt[:],
                    wqk_sb[:, jc, kc, :],
                    xT_sb[:, kc, c0:c0 + w],
                    start=(kc == 0), stop=(kc == KC - 1),
                )
            clock["pe"] += KC * w * CYC
            nc.vector.tensor_copy(qk_sb[:, jc, c0:c0 + w], pt[:])

        def v_chunk(mc, h):
            # two key-chunks in one psum tile (single bank-accumulation
            # group: start zeroes the bank once), one eviction
            pt = psum.tile([P, 2, 64], f32, tag="S", bufs=3,
                           name=f"vp_{mc}_{h}")
            for kc in range(KC):
                for sub in range(2):
                    nc.tensor.matmul(
                        pt[:, sub, :],
                        xT_sb[:, kc, (mc + sub) * P:(mc + sub + 1) * P],
                        wv_sb[:, kc, h * 64:(h + 1) * 64],
                        start=(kc == 0 and sub == 0),
                        stop=(kc == KC - 1 and sub == 1),
                    )
            clock["pe"] += KC * 128 * CYC
            nc.vector.tensor_copy(v_sb[:, mc:mc + 2, h, 0:64], pt[:])

        ot_sb = {}

        def get_ot(t):
            if t not in ot_sb:
                ot_sb[t] = att.tile([P, D], f16, tag=f"ot{t % 3}", bufs=2,
                                    name=f"ot_{t}")
            return ot_sb[t]

        def out_half(t, oc, ev="dve"):
            po = psum.tile([P, 512], f32, tag="S", bufs=3, name=f"po_{t}_{oc}")
            for dc in range(2):
                nc.tensor.matmul(
                    po[:],
                    zn_sb[:, dc, t * P:(t + 1) * P],
                    wo_sb[:, dc, oc * 512:(oc + 1) * 512],
                    start=(dc == 0), stop=(dc == 1),
                )
            clock["pe"] += 1024 * CYC
            ot = get_ot(t)
            if ev == "act":
                nc.scalar.copy(ot[:, oc * 512:(oc + 1) * 512], po[:])
                clock["act"] += 512 * 0.8333 + 185
            else:
                nc.vector.tensor_copy(ot[:, oc * 512:(oc + 1) * 512], po[:])
            if oc == 1:
                nc.sync.dma_start(out_ap[:, t, :], ot[:])
                ot_sb.pop(t)

        # ---- fill queue: [deadline (bi, slot), ready (bi, slot), cost, fn] ----
        fillq = []

        def add_fill(deadline, fn, ready=(0, 0), cost=427):
            fillq.append([deadline, ready, cost, fn])

        def drain_fills(bi, slot, blk_pe0, blk_act0):
            now = (bi, slot)
            due = [f for f in fillq if f[0] <= now]
            for f in due:
                fillq.remove(f)
                f[3]()
            # optional pulls: keep block-cumulative PE below the exp cadence
            while True:
                credit = (clock["act"] - blk_act0) - (clock["pe"] - blk_pe0)
                ok = [f for f in fillq if f[1] <= now and f[2] <= credit]
                if not ok:
                    break
                f = min(ok, key=lambda f: f[0])
                fillq.remove(f)
                f[3]()

        # ones column 64 for every head — keeps softmax row-sums on psum
        # partitions 0-63 where the custom-DVE reciprocal is valid.
        ones_sc = pers.tile([P, 64], f32)
        nc.vector.memset(ones_sc[:], 1.0)
        for h in range(HPC):
            nc.vector.tensor_copy(
                v_sb[:, :, h, 64:65],
                ones_sc[:, None, 0:1].to_broadcast((P, 16, 1)),
            )

        # ---- prologue: q0 0:1024 (two psum tiles, in DMA-arrival order)
        # + k0 chunk 0, so B0 (h0, q 0:1024) can start S(0) early
        for half in range(2):
            pq = psum.tile([P, 512], f32, tag="S", bufs=3,
                           name=f"qkp_0_pro{half}")
            for kc in range(KC):
                nc.tensor.matmul(
                    pq[:],
                    wqk_sb[:, 0, kc, :],
                    xT_sb[:, kc, 512 * half:512 * half + 512],
                    start=(kc == 0), stop=(kc == KC - 1),
                )
            clock["pe"] += 8 * 512 * CYC
            nc.vector.tensor_copy(qk_sb[:, 0, 512 * half:512 * half + 512], pq[:])
        qk_chunk(2, 0)

        # ---- single-head attention block; tail deferred into next block ----
        AV_LAG = 4
        pending = []          # closure lists from the previous block's tail

        trp_ev = [0]

        def tr_emit(hp2, t, mode):
            if mode == "dual":
                nc.sync.dma_start_transpose(
                    zn_sb[:, :, t * P:(t + 1) * P], ztm_sb[:, t, :, :])
            elif mode in ("pe", "pe-dve", "pe-act"):
                # PE transpose (out = ztm_slice^T via identity moving operand)
                # + ACT/DVE eviction: keeps the end region off the busy HWDGE
                trp = psum.tile([P, P], f32, tag="S", bufs=3,
                                name=f"trp_{t}_{hp2}")
                nc.tensor.matmul(trp[:], ztm_sb[:, t, hp2, :], id_sb[:],
                                 start=True, stop=True)
                clock["pe"] += P * CYC
                dst = zn_sb[:, hp2, t * P:(t + 1) * P]
                if mode == "pe-act" or (mode == "pe" and trp_ev[0] % 2):
                    nc.scalar.copy(dst, trp[:])
                else:
                    nc.vector.tensor_copy(dst, trp[:])
                trp_ev[0] += 1
            else:
                nc.sync.dma_start_transpose(
                    zn_sb[:, hp2, t * P:(t + 1) * P], ztm_sb[:, t, hp2, :])

        def attn_block(bi, h, l0, qn):
            hp = h // 2
            r0 = (h % 2) * 64
            nq2 = (qn + 511) // 512
            nqc = qn // P
            t0 = l0 // P
            zt = psum.tile([P, nqc, P], f32, tag="z", bufs=1, name=f"z_{h}_{l0}")
            Es = {}

            def do_av(mc):
                E = Es.pop(mc)
                for qc in range(nqc):
                    nc.tensor.matmul(
                        zt[:, qc, 0:65],
                        E[:, qc * P:(qc + 1) * P],
                        v_sb[:, mc, h, 0:65],
                        start=(mc == 0 and qc % 4 == 0),
                        stop=(mc == 15 and (qc % 4 == 3 or qc == nqc - 1)),
                    )
                clock["pe"] += nqc * 65 * CYC

            blk_pe0, blk_act0 = clock["pe"], clock["act"]
            grp = 4 if qn <= 256 else 1
            Sgrp = [None]
            for mc in range(16):
                if grp > 1:
                    # several key-chunks share one psum tile and one exp
                    if mc % grp == 0:
                        Sgrp[0] = psum.tile([P, grp, qn], f32, tag="S", bufs=3,
                                            name=f"S_{h}_{l0}_{mc}")
                    S = Sgrp[0][:, mc % grp, :]
                else:
                    S = psum.tile([P, qn], f32, tag="S", bufs=3,
                                  name=f"S_{h}_{l0}_{mc}")[:]
                for q2 in range(nq2):
                    w = min(512, qn - q2 * 512)
                    nc.tensor.matmul(
                        S[:, q2 * 512:q2 * 512 + w],
                        qk_sb[r0:r0 + 64, 2 + hp, mc * P:(mc + 1) * P],
                        qk_sb[r0:r0 + 64, hp, l0 + q2 * 512:l0 + q2 * 512 + w],
                        start=True, stop=True,
                    )
                clock["pe"] += qn * CYC
                if grp > 1:
                    if mc % grp == grp - 1:
                        E2 = att.tile([P, grp, qn], f16, tag="E", bufs=8,
                                      name=f"E_{h}_{l0}_{mc}")
                        nc.scalar.activation(E2[:], Sgrp[0][:], Exp, scale=0.125)
                        clock["act"] += grp * qn * 0.8333 + 185
                        for i in range(grp):
                            Es[mc - grp + 1 + i] = E2[:, i, :]
                else:
                    E = att.tile([P, qn], f16, tag="E", bufs=8,
                                 name=f"E_{h}_{l0}_{mc}")
                    nc.scalar.activation(E[:], S[:], Exp, scale=0.125)
                    clock["act"] += qn * 0.8333 + 185
                    Es[mc] = E
                for fn in (pending[mc] if mc < len(pending) else ()):
                    fn()
                if mc >= AV_LAG:
                    do_av(mc - AV_LAG)
                drain_fills(bi, mc, blk_pe0, blk_act0)
            for fns in pending[16:]:
                for fn in fns:
                    fn()
            del pending[:]

            def norm_and_tr():
                rsb = att.tile([P, 16], f32, tag="r", bufs=2, name=f"r_{h}_{l0}")
                nc.vector.reciprocal_approx_fast(out=rsb[:, 0:nqc], in_=zt[:, :, 64])
                trs = tr_plan.pop(bi, ())
                # per-chunk normalize when this block's chunks gate its own
                # transposes (shortens the zt -> zn -> out critical path)
                percnk = [x for x in trs if t0 <= x[1] < t0 + nqc] if bi >= 7 else []
                if percnk:
                    done = set()
                    for hp2, t, mode in percnk:
                        nc.vector.tensor_tensor(
                            ztm_sb[:, t, hp, r0:r0 + 64], zt[:, t - t0, 0:64],
                            rsb[:, t - t0, None].to_broadcast((P, 64)), Mult,
                        )
                        done.add(t)
                        tr_emit(hp2, t, mode)
                    rest = [c for c in range(nqc) if t0 + c not in done]
                    if rest:
                        for c in rest:
                            nc.vector.tensor_tensor(
                                ztm_sb[:, t0 + c, hp, r0:r0 + 64],
                                zt[:, c, 0:64],
                                rsb[:, c, None].to_broadcast((P, 64)), Mult,
                            )
                else:
                    nc.vector.tensor_tensor(
                        ztm_sb[:, t0:t0 + nqc, hp, r0:r0 + 64], zt[:, :, 0:64],
                        rsb[:, 0:nqc, None].to_broadcast((P, nqc, 64)), Mult,
                    )
                    for hp2, t, mode in trs:
                        tr_emit(hp2, t, mode)

            pending.append([lambda: do_av(12), lambda: do_av(13)])
            pending.append([lambda: do_av(14)])
            pending.append([lambda: do_av(15)])
            pending.append([norm_and_tr])

        # transposes issued after block bi's norm: (pair, chunk, dual)
        tr_plan = {
            3: [(0, t, "dma") for t in range(8, 16)],
            5: [(None, t, "dual") for t in range(8)],
            7: [(1, t, "pe") for t in range(8, 14)],
            8: [(1, t, "pe") for t in range(14, 16)],
        }

        # ---- fill schedule ----
        # k pair0 chunks 1..15 JIT for B0 (prologue made chunk 0);
        # 256-wide pieces halve the psum-slot churn and DVE evictions
        add_fill((0, 0), lambda: qk_chunk(2, 128))
        for c in (2, 6, 10):
            add_fill((0, max(0, c - 2)),
                     lambda c=c: qk_chunk(2, 128 * c, w=512), cost=1707)
        add_fill((0, 12), lambda: qk_chunk(2, 128 * 14, w=256), cost=854)
        # v heads JIT before their first blocks (B0, B1, B4, B5);
        # paired key-chunks per fill
        for h, bi in ((0, 0), (1, 1), (2, 4), (3, 5)):
            for m in range(0, 16, 2):
                add_fill((bi, min(m, 15)), lambda m=m, h=h: v_chunk(m, h),
                         cost=427)
        # q pair0 second half (for B2 h0 1024:2048); xT cols land mid-B0
        for i in range(4):
            add_fill((1, 4 + 2 * i),
                     lambda c0=1024 + 256 * i: qk_chunk(0, c0, w=256),
                     ready=(0, 8 + 2 * i), cost=854)
        # q pair1 first half (for B4 h2 / B5 h3 0:1024)
        for i in range(4):
            add_fill((2, 4 + 2 * i), lambda c0=256 * i: qk_chunk(1, c0, w=256),
                     ready=(0, 8 + 2 * i), cost=854)
        # q pair1 second half (for B6 h2 1024:2048)
        for i in range(4):
            add_fill((5, 4 + 2 * i),
                     lambda c0=1024 + 256 * i: qk_chunk(1, c0, w=256),
                     ready=(5, 0), cost=854)
        # k pair1 chunks: 0,1 before B4; rest JIT inside B4 (512-wide)
        add_fill((3, 6), lambda: qk_chunk(3, 0))
        add_fill((3, 8), lambda: qk_chunk(3, 128))
        for c in range(2, 16, 2):
            add_fill((4, max(0, c - 2)),
                     lambda c=c: qk_chunk(3, 128 * c, w=256), cost=854)
        # out t0..7 (zn chunks 0:8 dual-transposed from B6 slot ~5);
        # spread across the ACT-bound blocks B6..B7
        for i in range(16):
            t, oc = i // 2, i % 2
            idx = 6 + (41 * i) // 16
            dl = (6 + idx // 16, idx % 16)
            add_fill(dl, lambda t=t, oc=oc: out_half(t, oc), ready=dl)
        # out t8..13 (zn pair0 8:16 after B3 norm; pair1 8:14 PE-transposed
        # from B8 slot ~5): forced in B8, evictions alternate ACT / DVE
        for i in range(12):
            t, oc = 8 + i // 2, i % 2
            dl = (8, 4 + (11 * i) // 12)
            ev = "act" if i % 2 else "dve"
            add_fill(dl, lambda t=t, oc=oc, ev=ev: out_half(t, oc, ev),
                     ready=dl)
        # ---- block sequence ----
        attn_block(0, 0, 0, 1024)
        attn_block(1, 1, 0, 1024)
        attn_block(2, 0, 1024, 1024)
        attn_block(3, 1, 1024, 1024)
        attn_block(4, 2, 0, 1024)
        attn_block(5, 3, 0, 1024)
        attn_block(6, 2, 1024, 1024)
        attn_block(7, 3, 1024, 768)
        attn_block(8, 3, 1792, 256)

        # tail: flush deferred AVs + norm + transposes, then out t14, t15
        # (per-oc pieces, adds alternating DVE / GpSimd, per-piece DMAs)
        for fns in pending:
            for fn in fns:
                fn()
        del pending[:]
        assert not fillq, f"unemitted fills: {len(fillq)}"
        for t in (14, 15):
            ot = att.tile([P, D], f16, tag=f"tl{t % 2}", bufs=1, name=f"otl_{t}")
            for oc in range(2):
                po = psum.tile([P, 512], f32, tag="S", bufs=3, name=f"pot_{t}_{oc}")
                for dc in range(2):
                    nc.tensor.matmul(
                        po[:],
                        zn_sb[:, dc, t * P:(t + 1) * P],
                        wo_sb[:, dc, oc * 512:(oc + 1) * 512],
                        start=(dc == 0), stop=(dc == 1),
                    )
                if oc == 0:
                    nc.scalar.copy(ot[:, 0:512], po[:])
                else:
                    nc.vector.tensor_copy(ot[:, 512:1024], po[:])
            nc.sync.dma_start(out_ap[:, t, :], ot[:])

    nc.compile()
    return nc


def _get_compiled():
    global _COMPILED
    if _COMPILED is None:
        _COMPILED = _build()
    return _COMPILED


def _shard_inputs(x, W_in, W_out):
    in_maps = []
    xTs = [x[:, b, :].T.astype(np.float16) for b in range(B)]
    for c in range(NCORES):
        b = c // 4
        lo = (c % 4) * J
        Wq = W_in[lo:lo + J]
        Wk = W_in[D + lo:D + lo + J]
        Wv = W_in[2 * D + lo:2 * D + lo + J]
        in_maps.append({
            "xT": xTs[b],
            "wqkT": np.ascontiguousarray(
                np.concatenate([Wq, Wk], 0).T.reshape(KC, P, 4, P)
                .transpose(2, 1, 0, 3).reshape(2 * J, KC * P)
            ).astype(np.float16),
            "wvT": Wv.T.astype(np.float16),
            "woT": np.ascontiguousarray(W_out[:, lo:lo + J].T).astype(np.float16),
            "ident": np.eye(P, dtype=np.float16),
        })
    return in_maps


def _reference_numpy(q, mask, W_in, b_in, W_out, b_out, num_heads):
    l, b, d = q.shape
    hd = d // num_heads
    qkv = q.reshape(l * b, d) @ W_in.T + b_in
    qkv = qkv.reshape(l, b, 3 * d)
    qh, kh, vh = np.split(qkv, 3, axis=-1)

    def to_heads(t):
        return t.reshape(l, b * num_heads, hd).transpose(1, 0, 2)

    qh, kh, vh = to_heads(qh), to_heads(kh), to_heads(vh)
    qh = qh / np.sqrt(np.float32(hd))
    scores = np.einsum("nld,nmd->nlm", qh, kh) + mask
    scores -= scores.max(axis=-1, keepdims=True)
    e = np.exp(scores)
    attn = e / e.sum(axis=-1, keepdims=True)
    z = np.einsum("nlm,nmd->nld", attn, vh)
    z = z.transpose(1, 0, 2).reshape(l * b, d)
    z = z @ W_out.T + b_out
    return z.reshape(l, b, d).astype(np.float32)


def kernel(q, k, v, mask, W_in, b_in, W_out, b_out, num_heads):
    num_heads = int(num_heads)
    q = np.asarray(q, dtype=np.float32)
    W_in = np.asarray(W_in, dtype=np.float32)
    W_out = np.asarray(W_out, dtype=np.float32)
    b_in = np.asarray(b_in, dtype=np.float32)
    b_out = np.asarray(b_out, dtype=np.float32)
    mask = np.asarray(mask, dtype=np.float32)

    if (
        num_heads != H
        or q.shape != (L, B, D)
        or W_in.shape != (3 * D, D)
        or W_out.shape != (D, D)
        or np.any(mask)
        or np.any(b_in)
    ):
        return _reference_numpy(q, mask, W_in, b_in, W_out, b_out, num_heads)

    from concourse import bass_utils

    nc = _get_compiled()
    in_maps = _shard_inputs(q, W_in, W_out)
    res = bass_utils.run_bass_kernel_spmd(
        nc, in_maps, core_ids=list(range(NCORES))
    )

    out = np.zeros((L, B, D), dtype=np.float32)
    for c in range(NCORES):
        out[:, c // 4, :] += res.results[c]["out_p"].astype(np.float32)
    out += b_out
    return out


# revision 111
# speedup vs baseline: 1.0047x; 1.0047x over previous
"""Multi-head attention layer (L=2048, B=2, D=1024, H=16) on 8 Trainium2 cores.

Sharding: batch*heads across cores — core c handles batch c//4, heads
4*(c%4)..4*(c%4)+4.  Tensor-parallel W_in column slice (per-head) and W_out
row slice; per-core partial outputs are summed on the host (2 groups of 4).

v33: deadline-scheduled fill queue.  The kernel is a sequence of ten
single-head attention blocks (S = K^T Q per key-chunk mc, exp on ACT,
AV accumulate, softmax-normalize).  All projection / output work is a
queue of fill closures with (block, slot) deadlines; each mc slot first
emits every due fill, then pulls optional fills while emitted PE time
trails emitted ACT time, keeping both engines saturated.  Block tails
(last AVs, normalize, z transposes) are deferred into the next block's
early slots so the PE crosses block boundaries without draining.
z is accumulated token-major (65-wide AV with a ones column for row
sums), normalized into a persistent [q, chunk, pair, j] buffer, and
moved to dim-major via DMA-engine XBAR transposes (off PE/DVE/ACT).
All matmul operands f16; f16 output DMA; b_out added on host.
"""

import sys

for _p in ("/opt/trn_rl_repo",):
    if _p not in sys.path:
        sys.path.append(_p)

import numpy as np

L, B, D, H = 2048, 2, 1024, 16
HD = 64
NCORES = 8
HPC = 4              # heads per core
J = HPC * HD         # 256 per-core head-dim slice
KC = D // 128        # 8 contraction chunks
P = 128

_COMPILED = None


def _build():
    import concourse.bacc as bacc
    import concourse.mybir as mybir
    import concourse.tile as tile
    from contextlib import ExitStack

    f32 = mybir.dt.float32
    f16 = mybir.dt.float16
    Exp = mybir.ActivationFunctionType.Exp
    Mult = mybir.AluOpType.mult
    Add = mybir.AluOpType.add

    nc = bacc.Bacc("TRN2", target_bir_lowering=False, debug=False)

    xT_d = nc.dram_tensor("xT", (D, L), f16, kind="ExternalInput")
    wqk_d = nc.dram_tensor("wqkT", (2 * J, KC * P), f16, kind="ExternalInput")
    wv_d = nc.dram_tensor("wvT", (D, J), f16, kind="ExternalInput")
    wo_d = nc.dram_tensor("woT", (J, D), f16, kind="ExternalInput")
    ident_d = nc.dram_tensor("ident", (P, P), f16, kind="ExternalInput")
    out_d = nc.dram_tensor("out_p", (L, D), f16, kind="ExternalOutput")

    with tile.TileContext(nc) as tc, ExitStack() as ctx:
        pers = ctx.enter_context(tc.tile_pool(name="pers", bufs=1))
        psum = ctx.enter_context(tc.tile_pool(name="psum", bufs=2, space="PSUM"))
        att = ctx.enter_context(tc.tile_pool(name="att", bufs=3))

        qk_sb = pers.tile([P, 4, L], f16)           # jc 0,1: q pairs; 2,3: k pairs
        v_sb = pers.tile([P, 16, HPC, P], f16)      # v cols 0:64, ones col 64
        zn_sb = pers.tile([P, 2, L], f16)           # dim-major normalized z per pair
        ztm_sb = pers.tile([P, 16, 2, P], f16)      # token-major z [q, chunk, pair, j]
        wo_sb = pers.tile([P, 2, D], f16)
        xT_sb = pers.tile([P, KC, L], f16)
        wqk_sb = pers.tile([P, 4, KC, P], f16)
        wv_sb = pers.tile([P, KC, J], f16)
        id_sb = pers.tile([P, P], f16)

        # PE warm-up: a chain of dummy matmuls keeps the PE busy through the
        # initial DMA window so the pstate ramp elapses before real work
        # (an idle gap resets pe_busy_start).
        warm = pers.tile([P, 512], f16)
        nc.vector.memset(warm[:], 0.0)
        wp = psum.tile([P, 512], f32, tag="z", bufs=1, name="warm")
        for _ in range(6):
            nc.tensor.matmul(wp[0:1, :], warm[:, 0:1], warm[:], start=True,
                             stop=True)

        out_ap = out_d.ap().rearrange("(t p) o -> p t o", p=P)
        xT_ap = xT_d.ap().rearrange("(kc p) m -> p kc m", p=P)
        wqk_ap = wqk_d.ap().rearrange("(jc p) f -> p jc f", p=P)
        wv_ap = wv_d.ap().rearrange("(kc p) j -> p kc j", p=P)
        wo_ap = wo_d.ap().rearrange("(dc p) o -> p dc o", p=P)

        # stripe DMAs ordered for earliest prologue start
        nc.sync.dma_start(wqk_sb[:, 0].rearrange("p kc c -> p (kc c)"), wqk_ap[:, 0])
        nc.scalar.dma_start(xT_sb[:, 0:4, 0:512], xT_ap[:, 0:4, 0:512])
        nc.sync.dma_start(wqk_sb[:, 2].rearrange("p kc c -> p (kc c)"), wqk_ap[:, 2])
        nc.scalar.dma_start(xT_sb[:, 4:8, 0:512], xT_ap[:, 4:8, 0:512])
        nc.sync.dma_start(wv_sb[:], wv_ap[:])
        nc.scalar.dma_start(xT_sb[:, 0:4, 512:1024], xT_ap[:, 0:4, 512:1024])
        nc.sync.dma_start(xT_sb[:, 4:8, 512:1024], xT_ap[:, 4:8, 512:1024])
        nc.scalar.dma_start(wqk_sb[:, 1].rearrange("p kc c -> p (kc c)"), wqk_ap[:, 1])
        nc.sync.dma_start(wqk_sb[:, 3].rearrange("p kc c -> p (kc c)"), wqk_ap[:, 3])
        nc.scalar.dma_start(xT_sb[:, 0:4, 1024:1536], xT_ap[:, 0:4, 1024:1536])
        nc.sync.dma_start(xT_sb[:, 4:8, 1024:1536], xT_ap[:, 4:8, 1024:1536])
        nc.scalar.dma_start(xT_sb[:, 0:4, 1536:2048], xT_ap[:, 0:4, 1536:2048])
        nc.sync.dma_start(xT_sb[:, 4:8, 1536:2048], xT_ap[:, 4:8, 1536:2048])
        nc.scalar.dma_start(wo_sb[:], wo_ap[:])

        # ---- emitted-work clocks (ns) for greedy fill balancing ----
        clock = {"pe": 0.0, "act": 0.0}
        CYC = 0.4167

        # ---- fill primitives ----
        def qk_chunk(jc, c0, w=128):
            pt = psum.tile([P, w], f32, tag="S", bufs=3, name=f"qkp_{jc}_{c0}")
            for kc in range(KC):
                nc.tensor.matmul(
                    pt[:],
                    wqk_sb[:, jc, kc, :],
                    xT_sb[:, kc, c0:c0 + w],
                    start=(kc == 0), stop=(kc == KC - 1),
                )
            clock["pe"] += KC * w * CYC
            nc.vector.tensor_copy(qk_sb[:, jc, c0:c0 + w], pt[:])

        def v_chunk(mc, h):
            # two key-chunks in one psum tile (single bank-accumulation
            # group: start zeroes the bank once), one eviction
            pt = psum.tile([P, 2, 64], f32, tag="S", bufs=3,
                           name=f"vp_{mc}_{h}")
            for kc in range(KC):
                for sub in range(2):
                    nc.tensor.matmul(
                        pt[:, sub, :],
                        xT_sb[:, kc, (mc + sub) * P:(mc + sub + 1) * P],
                        wv_sb[:, kc, h * 64:(h + 1) * 64],
                        start=(kc == 0 and sub == 0),
                        stop=(kc == KC - 1 and sub == 1),
                    )
            clock["pe"] += KC * 128 * CYC
            nc.vector.tensor_copy(v_sb[:, mc:mc + 2, h, 0:64], pt[:])

        ot_sb = {}

        def get_ot(t):
            if t not in ot_sb:
                ot_sb[t] = att.tile([P, D], f16, tag=f"ot{t % 3}", bufs=2,
                                    name=f"ot_{t}")
            return ot_sb[t]

        def out_half(t, oc, ev="dve"):
            po = psum.tile([P, 512], f32, tag="S", bufs=3, name=f"po_{t}_{oc}")
            for dc in range(2):
                nc.tensor.matmul(
                    po[:],
                    zn_sb[:, dc, t * P:(t + 1) * P],
                    wo_sb[:, dc, oc * 512:(oc + 1) * 512],
                    start=(dc == 0), stop=(dc == 1),
                )
            clock["pe"] += 1024 * CYC
            ot = get_ot(t)
            if ev == "act":
                nc.scalar.copy(ot[:, oc * 512:(oc + 1) * 512], po[:])
                clock["act"] += 512 * 0.8333 + 185
            else:
                nc.vector.tensor_copy(ot[:, oc * 512:(oc + 1) * 512], po[:])
            if oc == 1:
                nc.sync.dma_start(out_ap[:, t, :], ot[:])
                ot_sb.pop(t)

        # ---- fill queue: [deadline (bi, slot), ready (bi, slot), cost, fn] ----
        fillq = []

        def add_fill(deadline, fn, ready=(0, 0), cost=427):
            fillq.append([deadline, ready, cost, fn])

        def drain_fills(bi, slot, blk_pe0, blk_act0):
            now = (bi, slot)
            due = [f for f in fillq if f[0] <= now]
            for f in due:
                fillq.remove(f)
                f[3]()
            # optional pulls: keep block-cumulative PE below the exp cadence
            while True:
                credit = (clock["act"] - blk_act0) - (clock["pe"] - blk_pe0)
                ok = [f for f in fillq if f[1] <= now and f[2] <= credit]
                if not ok:
                    break
                f = min(ok, key=lambda f: f[0])
                fillq.remove(f)
                f[3]()

        # ones column 64 for every head — keeps softmax row-sums on psum
        # partitions 0-63 where the custom-DVE reciprocal is valid.
        ones_sc = pers.tile([P, 64], f32)
        nc.vector.memset(ones_sc[:], 1.0)
        for h in range(HPC):
            nc.vector.tensor_copy(
                v_sb[:, :, h, 64:65],
                ones_sc[:, None, 0:1].to_broadcast((P, 16, 1)),
            )

        # ---- prologue: q0 0:1024 (two psum tiles, in DMA-arrival order)
        # + k0 chunk 0, so B0 (h0, q 0:1024) can start S(0) early
        for half in range(2):
            pq = psum.tile([P, 512], f32, tag="S", bufs=3,
                           name=f"qkp_0_pro{half}")
            for kc in range(KC):
                nc.tensor.matmul(
                    pq[:],
                    wqk_sb[:, 0, kc, :],
                    xT_sb[:, kc, 512 * half:512 * half + 512],
                    start=(kc == 0), stop=(kc == KC - 1),
                )
            clock["pe"] += 8 * 512 * CYC
            nc.vector.tensor_copy(qk_sb[:, 0, 512 * half:512 * half + 512], pq[:])
        qk_chunk(2, 0)

        # ---- single-head attention block; tail deferred into next block ----
        AV_LAG = 4
        pending = []          # closure lists from the previous block's tail

        trp_ev = [0]

        def tr_emit(hp2, t, mode):
            if mode == "dual":
                nc.sync.dma_start_transpose(
                    zn_sb[:, :, t * P:(t + 1) * P], ztm_sb[:, t, :, :])
            elif mode in ("pe", "pe-dve", "pe-act"):
                # PE transpose (out = ztm_slice^T via identity moving operand)
                # + ACT/DVE eviction: keeps the end region off the busy HWDGE
                trp = psum.tile([P, P], f32, tag="S", bufs=3,
                                name=f"trp_{t}_{hp2}")
                nc.tensor.matmul(trp[:], ztm_sb[:, t, hp2, :], id_sb[:],
                                 start=True, stop=True)
                clock["pe"] += P * CYC
                dst = zn_sb[:, hp2, t * P:(t + 1) * P]
                if mode == "pe-act" or (mode == "pe" and trp_ev[0] % 2):
                    nc.scalar.copy(dst, trp[:])
                else:
                    nc.vector.tensor_copy(dst, trp[:])
                trp_ev[0] += 1
            else:
                nc.sync.dma_start_transpose(
                    zn_sb[:, hp2, t * P:(t + 1) * P], ztm_sb[:, t, hp2, :])

        def attn_block(bi, h, l0, qn):
            hp = h // 2
            r0 = (h % 2) * 64
            nq2 = (qn + 511) // 512
            nqc = qn // P
            t0 = l0 // P
            zt = psum.tile([P, nqc, P], f32, tag="z", bufs=1, name=f"z_{h}_{l0}")
            Es = {}

            def do_av(mc):
                E = Es.pop(mc)
                for qc in range(nqc):
                    nc.tensor.matmul(
                        zt[:, qc, 0:65],
                        E[:, qc * P:(qc + 1) * P],
                        v_sb[:, mc, h, 0:65],
                        start=(mc == 0 and qc % 4 == 0),
                        stop=(mc == 15 and (qc % 4 == 3 or qc == nqc - 1)),
                    )
                clock["pe"] += nqc * 65 * CYC

            blk_pe0, blk_act0 = clock["pe"], clock["act"]
            grp = 4 if qn <= 256 else 1
            Sgrp = [None]
            for mc in range(16):
                if grp > 1:
                    # several key-chunks share one psum tile and one exp
                    if mc % grp == 0:
                        Sgrp[0] = psum.tile([P, grp, qn], f32, tag="S", bufs=3,
                                            name=f"S_{h}_{l0}_{mc}")
                    S = Sgrp[0][:, mc % grp, :]
                else:
                    S = psum.tile([P, qn], f32, tag="S", bufs=3,
                                  name=f"S_{h}_{l0}_{mc}")[:]
                for q2 in range(nq2):
                    w = min(512, qn - q2 * 512)
                    nc.tensor.matmul(
                        S[:, q2 * 512:q2 * 512 + w],
                        qk_sb[r0:r0 + 64, 2 + hp, mc * P:(mc + 1) * P],
                        qk_sb[r0:r0 + 64, hp, l0 + q2 * 512:l0 + q2 * 512 + w],
                        start=True, stop=True,
                    )
                clock["pe"] += qn * CYC
                if grp > 1:
                    if mc % grp == grp - 1:
                        E2 = att.tile([P, grp, qn], f16, tag="E", bufs=8,
                                      name=f"E_{h}_{l0}_{mc}")
                        nc.scalar.activation(E2[:], Sgrp[0][:], Exp, scale=0.125)
                        clock["act"] += grp * qn * 0.8333 + 185
                        for i in range(grp):
                            Es[mc - grp + 1 + i] = E2[:, i, :]
                else:
                    E = att.tile([P, qn], f16, tag="E", bufs=8,
                                 name=f"E_{h}_{l0}_{mc}")
                    nc.scalar.activation(E[:], S[:], Exp, scale=0.125)
                    clock["act"] += qn * 0.8333 + 185
                    Es[mc] = E
                for fn in (pending[mc] if mc < len(pending) else ()):
                    fn()
                if mc >= AV_LAG:
                    do_av(mc - AV_LAG)
                drain_fills(bi, mc, blk_pe0, blk_act0)
            for fns in pending[16:]:
                for fn in fns:
                    fn()
            del pending[:]

            def norm_and_tr():
                rsb = att.tile([P, 16], f32, tag="r", bufs=2, name=f"r_{h}_{l0}")
                nc.vector.reciprocal_approx_fast(out=rsb[:, 0:nqc], in_=zt[:, :, 64])
                trs = tr_plan.pop(bi, ())
                # per-chunk normalize when this block's chunks gate its own
                # transposes (shortens the zt -> zn -> out critical path)
                percnk = [x for x in trs if t0 <= x[1] < t0 + nqc] if bi >= 7 else []
                if percnk:
                    done = set()
                    for hp2, t, mode in percnk:
                        nc.vector.tensor_tensor(
                            ztm_sb[:, t, hp, r0:r0 + 64], zt[:, t - t0, 0:64],
                            rsb[:, t - t0, None].to_broadcast((P, 64)), Mult,
                        )
                        done.add(t)
                        tr_emit(hp2, t, mode)
                    rest = [c for c in range(nqc) if t0 + c not in done]
                    if rest:
                        for c in rest:
                            nc.vector.tensor_tensor(
                                ztm_sb[:, t0 + c, hp, r0:r0 + 64],
                                zt[:, c, 0:64],
                                rsb[:, c, None].to_broadcast((P, 64)), Mult,
                            )
                else:
                    nc.vector.tensor_tensor(
                        ztm_sb[:, t0:t0 + nqc, hp, r0:r0 + 64], zt[:, :, 0:64],
                        rsb[:, 0:nqc, None].to_broadcast((P, nqc, 64)), Mult,
                    )
                    for hp2, t, mode in trs:
                        tr_emit(hp2, t, mode)

            pending.append([lambda: do_av(12), lambda: do_av(13)])
            pending.append([lambda: do_av(14)])
            pending.append([lambda: do_av(15)])
            pending.append([norm_and_tr])

        # transposes issued after block bi's norm: (pair, chunk, dual)
        tr_plan = {
            3: [(0, t, "dma") for t in range(8, 16)],
            5: [(None, t, "dual") for t in range(8)],
            7: [(1, t, "pe") for t in range(8, 14)],
            8: [(1, t, "pe") for t in range(14, 16)],
        }

        # ---- fill schedule ----
        # k pair0 chunks 1..15 JIT for B0 (prologue made chunk 0);
        # 256-wide pieces halve the psum-slot churn and DVE evictions
        add_fill((0, 0), lambda: qk_chunk(2, 128))
        for c in (2, 6, 10):
            add_fill((0, max(0, c - 2)),
                     lambda c=c: qk_chunk(2, 128 * c, w=512), cost=1707)
        add_fill((0, 12), lambda: qk_chunk(2, 128 * 14, w=256), cost=854)
        # v heads JIT before their first blocks (B0, B1, B4, B5);
        # paired key-chunks per fill
        for h, bi in ((0, 0), (1, 1), (2, 4), (3, 5)):
            for m in range(0, 16, 2):
                add_fill((bi, min(m + 1, 15)), lambda m=m, h=h: v_chunk(m, h),
                         cost=427)
        # q pair0 second half (for B2 h0 1024:2048); xT cols land mid-B0
        for i in range(4):
            add_fill((1, 4 + 2 * i),
                     lambda c0=1024 + 256 * i: qk_chunk(0, c0, w=256),
                     ready=(0, 8 + 2 * i), cost=854)
        # q pair1 first half (for B4 h2 / B5 h3 0:1024)
        for i in range(4):
            add_fill((2, 4 + 2 * i), lambda c0=256 * i: qk_chunk(1, c0, w=256),
                     ready=(0, 8 + 2 * i), cost=854)
        # q pair1 second half (for B6 h2 1024:2048)
        for i in range(4):
            add_fill((5, 4 + 2 * i),
                     lambda c0=1024 + 256 * i: qk_chunk(1, c0, w=256),
                     ready=(5, 0), cost=854)
        # k pair1 chunks: 0,1 before B4; rest JIT inside B4 (512-wide)
        add_fill((3, 6), lambda: qk_chunk(3, 0))
        add_fill((3, 8), lambda: qk_chunk(3, 128))
        for c in range(2, 16, 2):
            add_fill((4, max(0, c - 2)),
                     lambda c=c: qk_chunk(3, 128 * c, w=256), cost=854)
        # out t0..7 (zn chunks 0:8 dual-transposed from B6 slot ~5);
        # spread across the ACT-bound blocks B6..B7
        for i in range(16):
            t, oc = i // 2, i % 2
            idx = 6 + (41 * i) // 16
            dl = (6 + idx // 16, idx % 16)
            add_fill(dl, lambda t=t, oc=oc: out_half(t, oc), ready=dl)
        # out t8..13 (zn pair0 8:16 after B3 norm; pair1 8:14 PE-transposed
        # from B8 slot ~5): forced in B8, evictions alternate ACT / DVE
        for i in range(12):
            t, oc = 8 + i // 2, i % 2
            dl = (8, 4 + (11 * i) // 12)
            ev = "act" if i % 2 else "dve"
            add_fill(dl, lambda t=t, oc=oc, ev=ev: out_half(t, oc, ev),
                     ready=dl)
        # ---- block sequence ----
        attn_block(0, 0, 0, 1024)
        attn_block(1, 1, 0, 1024)
        attn_block(2, 0, 1024, 1024)
        attn_block(3, 1, 1024, 1024)
        attn_block(4, 2, 0, 1024)
        attn_block(5, 3, 0, 1024)
        attn_block(6, 2, 1024, 1024)
        attn_block(7, 3, 1024, 768)
        attn_block(8, 3, 1792, 256)

        # tail: flush deferred AVs + norm + transposes, then out t14, t15
        # (per-oc pieces, adds alternating DVE / GpSimd, per-piece DMAs)
        for fns in pending:
            for fn in fns:
                fn()
        del pending[:]
        assert not fillq, f"unemitted fills: {len(fillq)}"
        for t in (14, 15):
            ot = att.tile([P, D], f16, tag=f"tl{t % 2}", bufs=1, name=f"otl_{t}")
            for oc in range(2):
                po = psum.tile([P, 512], f32, tag="S", bufs=3, name=f"pot_{t}_{oc}")
                for dc in range(2):
                    nc.tensor.matmul(
                        po[:],
                        zn_sb[:, dc, t * P:(t + 1) * P],
                        wo_sb[:, dc, oc * 512:(oc + 1) * 512],
                        start=(dc == 0), stop=(dc == 1),
                    )
                if oc == 0:
                    nc.scalar.copy(ot[:, 0:512], po[:])
                else:
                    nc.vector.tensor_copy(ot[:, 512:1024], po[:])
            nc.sync.dma_start(out_ap[:, t, :], ot[:])

    nc.compile()
    return nc


def _get_compiled():
    global _COMPILED
    if _COMPILED is None:
        _COMPILED = _build()
    return _COMPILED


def _shard_inputs(x, W_in, W_out):
    in_maps = []
    xTs = [x[:, b, :].T.astype(np.float16) for b in range(B)]
    for c in range(NCORES):
        b = c // 4
        lo = (c % 4) * J
        Wq = W_in[lo:lo + J]
        Wk = W_in[D + lo:D + lo + J]
        Wv = W_in[2 * D + lo:2 * D + lo + J]
        in_maps.append({
            "xT": xTs[b],
            "wqkT": np.ascontiguousarray(
                np.concatenate([Wq, Wk], 0).T.reshape(KC, P, 4, P)
                .transpose(2, 1, 0, 3).reshape(2 * J, KC * P)
            ).astype(np.float16),
            "wvT": Wv.T.astype(np.float16),
            "woT": np.ascontiguousarray(W_out[:, lo:lo + J].T).astype(np.float16),
            "ident": np.eye(P, dtype=np.float16),
        })
    return in_maps


def _reference_numpy(q, mask, W_in, b_in, W_out, b_out, num_heads):
    l, b, d = q.shape
    hd = d // num_heads
    qkv = q.reshape(l * b, d) @ W_in.T + b_in
    qkv = qkv.reshape(l, b, 3 * d)
    qh, kh, vh = np.split(qkv, 3, axis=-1)

    def to_heads(t):
        return t.reshape(l, b * num_heads, hd).transpose(1, 0, 2)

    qh, kh, vh = to_heads(qh), to_heads(kh), to_heads(vh)
    qh = qh / np.sqrt(np.float32(hd))
    scores = np.einsum("nld,nmd->nlm", qh, kh) + mask
    scores -= scores.max(axis=-1, keepdims=True)
    e = np.exp(scores)
    attn = e / e.sum(axis=-1, keepdims=True)
    z = np.einsum("nlm,nmd->nld", attn, vh)
    z = z.transpose(1, 0, 2).reshape(l * b, d)
    z = z @ W_out.T + b_out
    return z.reshape(l, b, d).astype(np.float32)


def kernel(q, k, v, mask, W_in, b_in, W_out, b_out, num_heads):
    num_heads = int(num_heads)
    q = np.asarray(q, dtype=np.float32)
    W_in = np.asarray(W_in, dtype=np.float32)
    W_out = np.asarray(W_out, dtype=np.float32)
    b_in = np.asarray(b_in, dtype=np.float32)
    b_out = np.asarray(b_out, dtype=np.float32)
    mask = np.asarray(mask, dtype=np.float32)

    if (
        num_heads != H
        or q.shape != (L, B, D)
        or W_in.shape != (3 * D, D)
        or W_out.shape != (D, D)
        or np.any(mask)
        or np.any(b_in)
    ):
        return _reference_numpy(q, mask, W_in, b_in, W_out, b_out, num_heads)

    from concourse import bass_utils

    nc = _get_compiled()
    in_maps = _shard_inputs(q, W_in, W_out)
    res = bass_utils.run_bass_kernel_spmd(
        nc, in_maps, core_ids=list(range(NCORES))
    )

    out = np.zeros((L, B, D), dtype=np.float32)
    for c in range(NCORES):
        out[:, c // 4, :] += res.results[c]["out_p"].astype(np.float32)
    out += b_out
    return out


# revision 112
# speedup vs baseline: 1.0080x; 1.0034x over previous
"""Multi-head attention layer (L=2048, B=2, D=1024, H=16) on 8 Trainium2 cores.

Sharding: batch*heads across cores — core c handles batch c//4, heads
4*(c%4)..4*(c%4)+4.  Tensor-parallel W_in column slice (per-head) and W_out
row slice; per-core partial outputs are summed on the host (2 groups of 4).

v33: deadline-scheduled fill queue.  The kernel is a sequence of ten
single-head attention blocks (S = K^T Q per key-chunk mc, exp on ACT,
AV accumulate, softmax-normalize).  All projection / output work is a
queue of fill closures with (block, slot) deadlines; each mc slot first
emits every due fill, then pulls optional fills while emitted PE time
trails emitted ACT time, keeping both engines saturated.  Block tails
(last AVs, normalize, z transposes) are deferred into the next block's
early slots so the PE crosses block boundaries without draining.
z is accumulated token-major (65-wide AV with a ones column for row
sums), normalized into a persistent [q, chunk, pair, j] buffer, and
moved to dim-major via DMA-engine XBAR transposes (off PE/DVE/ACT).
All matmul operands f16; f16 output DMA; b_out added on host.
"""

import sys

for _p in ("/opt/trn_rl_repo",):
    if _p not in sys.path:
        sys.path.append(_p)

import numpy as np

L, B, D, H = 2048, 2, 1024, 16
HD = 64
NCORES = 8
HPC = 4              # heads per core
J = HPC * HD         # 256 per-core head-dim slice
KC = D // 128        # 8 contraction chunks
P = 128

_COMPILED = None


def _build():
    import concourse.bacc as bacc
    import concourse.mybir as mybir
    import concourse.tile as tile
    from contextlib import ExitStack

    f32 = mybir.dt.float32
    f16 = mybir.dt.float16
    Exp = mybir.ActivationFunctionType.Exp
    Mult = mybir.AluOpType.mult
    Add = mybir.AluOpType.add

    nc = bacc.Bacc("TRN2", target_bir_lowering=False, debug=False)

    xT_d = nc.dram_tensor("xT", (D, L), f16, kind="ExternalInput")
    wqk_d = nc.dram_tensor("wqkT", (2 * J, KC * P), f16, kind="ExternalInput")
    wv_d = nc.dram_tensor("wvT", (D, J), f16, kind="ExternalInput")
    wo_d = nc.dram_tensor("woT", (J, D), f16, kind="ExternalInput")
    ident_d = nc.dram_tensor("ident", (P, P), f16, kind="ExternalInput")
    out_d = nc.dram_tensor("out_p", (L, D), f16, kind="ExternalOutput")

    with tile.TileContext(nc) as tc, ExitStack() as ctx:
        pers = ctx.enter_context(tc.tile_pool(name="pers", bufs=1))
        psum = ctx.enter_context(tc.tile_pool(name="psum", bufs=2, space="PSUM"))
        att = ctx.enter_context(tc.tile_pool(name="att", bufs=3))

        qk_sb = pers.tile([P, 4, L], f16)           # jc 0,1: q pairs; 2,3: k pairs
        v_sb = pers.tile([P, 16, HPC, P], f16)      # v cols 0:64, ones col 64
        zn_sb = pers.tile([P, 2, L], f16)           # dim-major normalized z per pair
        ztm_sb = pers.tile([P, 16, 2, P], f16)      # token-major z [q, chunk, pair, j]
        wo_sb = pers.tile([P, 2, D], f16)
        xT_sb = pers.tile([P, KC, L], f16)
        wqk_sb = pers.tile([P, 4, KC, P], f16)
        wv_sb = pers.tile([P, KC, J], f16)
        id_sb = pers.tile([P, P], f16)

        # PE warm-up: a chain of dummy matmuls keeps the PE busy through the
        # initial DMA window so the pstate ramp elapses before real work
        # (an idle gap resets pe_busy_start).
        warm = pers.tile([P, 512], f16)
        nc.vector.memset(warm[:], 0.0)
        wp = psum.tile([P, 512], f32, tag="z", bufs=1, name="warm")
        for _ in range(6):
            nc.tensor.matmul(wp[0:1, :], warm[:, 0:1], warm[:], start=True,
                             stop=True)

        out_ap = out_d.ap().rearrange("(t p) o -> p t o", p=P)
        xT_ap = xT_d.ap().rearrange("(kc p) m -> p kc m", p=P)
        wqk_ap = wqk_d.ap().rearrange("(jc p) f -> p jc f", p=P)
        wv_ap = wv_d.ap().rearrange("(kc p) j -> p kc j", p=P)
        wo_ap = wo_d.ap().rearrange("(dc p) o -> p dc o", p=P)

        # stripe DMAs ordered for earliest prologue start
        nc.sync.dma_start(wqk_sb[:, 0].rearrange("p kc c -> p (kc c)"), wqk_ap[:, 0])
        nc.scalar.dma_start(xT_sb[:, 0:4, 0:512], xT_ap[:, 0:4, 0:512])
        nc.sync.dma_start(wqk_sb[:, 2].rearrange("p kc c -> p (kc c)"), wqk_ap[:, 2])
        nc.scalar.dma_start(xT_sb[:, 4:8, 0:512], xT_ap[:, 4:8, 0:512])
        nc.sync.dma_start(wv_sb[:], wv_ap[:])
        nc.scalar.dma_start(xT_sb[:, 0:4, 512:1024], xT_ap[:, 0:4, 512:1024])
        nc.sync.dma_start(xT_sb[:, 4:8, 512:1024], xT_ap[:, 4:8, 512:1024])
        nc.scalar.dma_start(wqk_sb[:, 1].rearrange("p kc c -> p (kc c)"), wqk_ap[:, 1])
        nc.sync.dma_start(wqk_sb[:, 3].rearrange("p kc c -> p (kc c)"), wqk_ap[:, 3])
        nc.scalar.dma_start(xT_sb[:, 0:4, 1024:1536], xT_ap[:, 0:4, 1024:1536])
        nc.sync.dma_start(xT_sb[:, 4:8, 1024:1536], xT_ap[:, 4:8, 1024:1536])
        nc.scalar.dma_start(xT_sb[:, 0:4, 1536:2048], xT_ap[:, 0:4, 1536:2048])
        nc.sync.dma_start(xT_sb[:, 4:8, 1536:2048], xT_ap[:, 4:8, 1536:2048])
        nc.scalar.dma_start(wo_sb[:], wo_ap[:])

        # ---- emitted-work clocks (ns) for greedy fill balancing ----
        clock = {"pe": 0.0, "act": 0.0}
        CYC = 0.4167

        # ---- fill primitives ----
        def qk_chunk(jc, c0, w=128):
            pt = psum.tile([P, w], f32, tag="S", bufs=3, name=f"qkp_{jc}_{c0}")
            for kc in range(KC):
                nc.tensor.matmul(
                    pt[:],
                    wqk_sb[:, jc, kc, :],
                    xT_sb[:, kc, c0:c0 + w],
                    start=(kc == 0), stop=(kc == KC - 1),
                )
            clock["pe"] += KC * w * CYC
            nc.vector.tensor_copy(qk_sb[:, jc, c0:c0 + w], pt[:])

        def v_chunk(mc, h, nv=2):
            # nv key-chunks in one psum tile (single bank-accumulation
            # group: start zeroes the bank once), one eviction
            pt = psum.tile([P, nv, 64], f32, tag="S", bufs=3,
                           name=f"vp_{mc}_{h}")
            for kc in range(KC):
                for sub in range(nv):
                    nc.tensor.matmul(
                        pt[:, sub, :],
                        xT_sb[:, kc, (mc + sub) * P:(mc + sub + 1) * P],
                        wv_sb[:, kc, h * 64:(h + 1) * 64],
                        start=(kc == 0 and sub == 0),
                        stop=(kc == KC - 1 and sub == nv - 1),
                    )
            clock["pe"] += KC * nv * 64 * CYC
            nc.vector.tensor_copy(v_sb[:, mc:mc + nv, h, 0:64], pt[:])

        ot_sb = {}

        def get_ot(t):
            if t not in ot_sb:
                ot_sb[t] = att.tile([P, D], f16, tag=f"ot{t % 3}", bufs=2,
                                    name=f"ot_{t}")
            return ot_sb[t]

        def out_half(t, oc, ev="dve"):
            po = psum.tile([P, 512], f32, tag="S", bufs=3, name=f"po_{t}_{oc}")
            for dc in range(2):
                nc.tensor.matmul(
                    po[:],
                    zn_sb[:, dc, t * P:(t + 1) * P],
                    wo_sb[:, dc, oc * 512:(oc + 1) * 512],
                    start=(dc == 0), stop=(dc == 1),
                )
            clock["pe"] += 1024 * CYC
            ot = get_ot(t)
            if ev == "act":
                nc.scalar.copy(ot[:, oc * 512:(oc + 1) * 512], po[:])
                clock["act"] += 512 * 0.8333 + 185
            else:
                nc.vector.tensor_copy(ot[:, oc * 512:(oc + 1) * 512], po[:])
            if oc == 1:
                nc.sync.dma_start(out_ap[:, t, :], ot[:])
                ot_sb.pop(t)

        # ---- fill queue: [deadline (bi, slot), ready (bi, slot), cost, fn] ----
        fillq = []

        def add_fill(deadline, fn, ready=(0, 0), cost=427):
            fillq.append([deadline, ready, cost, fn])

        def drain_fills(bi, slot, blk_pe0, blk_act0):
            now = (bi, slot)
            due = [f for f in fillq if f[0] <= now]
            for f in due:
                fillq.remove(f)
                f[3]()
            # optional pulls: keep block-cumulative PE below the exp cadence
            while True:
                credit = (clock["act"] - blk_act0) - (clock["pe"] - blk_pe0)
                ok = [f for f in fillq if f[1] <= now and f[2] <= credit]
                if not ok:
                    break
                f = min(ok, key=lambda f: f[0])
                fillq.remove(f)
                f[3]()

        # ones column 64 for every head — keeps softmax row-sums on psum
        # partitions 0-63 where the custom-DVE reciprocal is valid.
        ones_sc = pers.tile([P, 64], f32)
        nc.vector.memset(ones_sc[:], 1.0)
        for h in range(HPC):
            nc.vector.tensor_copy(
                v_sb[:, :, h, 64:65],
                ones_sc[:, None, 0:1].to_broadcast((P, 16, 1)),
            )

        # ---- prologue: q0 0:1024 (two psum tiles, in DMA-arrival order)
        # + k0 chunk 0, so B0 (h0, q 0:1024) can start S(0) early
        for half in range(2):
            pq = psum.tile([P, 512], f32, tag="S", bufs=3,
                           name=f"qkp_0_pro{half}")
            for kc in range(KC):
                nc.tensor.matmul(
                    pq[:],
                    wqk_sb[:, 0, kc, :],
                    xT_sb[:, kc, 512 * half:512 * half + 512],
                    start=(kc == 0), stop=(kc == KC - 1),
                )
            clock["pe"] += 8 * 512 * CYC
            nc.vector.tensor_copy(qk_sb[:, 0, 512 * half:512 * half + 512], pq[:])
        qk_chunk(2, 0)

        # ---- single-head attention block; tail deferred into next block ----
        AV_LAG = 4
        pending = []          # closure lists from the previous block's tail

        trp_ev = [0]

        def tr_emit(hp2, t, mode):
            if mode == "dual":
                nc.sync.dma_start_transpose(
                    zn_sb[:, :, t * P:(t + 1) * P], ztm_sb[:, t, :, :])
            elif mode in ("pe", "pe-dve", "pe-act"):
                # PE transpose (out = ztm_slice^T via identity moving operand)
                # + ACT/DVE eviction: keeps the end region off the busy HWDGE
                trp = psum.tile([P, P], f32, tag="S", bufs=3,
                                name=f"trp_{t}_{hp2}")
                nc.tensor.matmul(trp[:], ztm_sb[:, t, hp2, :], id_sb[:],
                                 start=True, stop=True)
                clock["pe"] += P * CYC
                dst = zn_sb[:, hp2, t * P:(t + 1) * P]
                if mode == "pe-act" or (mode == "pe" and trp_ev[0] % 2):
                    nc.scalar.copy(dst, trp[:])
                else:
                    nc.vector.tensor_copy(dst, trp[:])
                trp_ev[0] += 1
            else:
                nc.sync.dma_start_transpose(
                    zn_sb[:, hp2, t * P:(t + 1) * P], ztm_sb[:, t, hp2, :])

        def attn_block(bi, h, l0, qn):
            hp = h // 2
            r0 = (h % 2) * 64
            nq2 = (qn + 511) // 512
            nqc = qn // P
            t0 = l0 // P
            zt = psum.tile([P, nqc, P], f32, tag="z", bufs=1, name=f"z_{h}_{l0}")
            Es = {}

            def do_av(mc):
                E = Es.pop(mc)
                for qc in range(nqc):
                    nc.tensor.matmul(
                        zt[:, qc, 0:65],
                        E[:, qc * P:(qc + 1) * P],
                        v_sb[:, mc, h, 0:65],
                        start=(mc == 0 and qc % 4 == 0),
                        stop=(mc == 15 and (qc % 4 == 3 or qc == nqc - 1)),
                    )
                clock["pe"] += nqc * 65 * CYC

            blk_pe0, blk_act0 = clock["pe"], clock["act"]
            grp = 4 if qn <= 256 else 1
            Sgrp = [None]
            for mc in range(16):
                if grp > 1:
                    # several key-chunks share one psum tile and one exp
                    if mc % grp == 0:
                        Sgrp[0] = psum.tile([P, grp, qn], f32, tag="S", bufs=3,
                                            name=f"S_{h}_{l0}_{mc}")
                    S = Sgrp[0][:, mc % grp, :]
                else:
                    S = psum.tile([P, qn], f32, tag="S", bufs=3,
                                  name=f"S_{h}_{l0}_{mc}")[:]
                for q2 in range(nq2):
                    w = min(512, qn - q2 * 512)
                    nc.tensor.matmul(
                        S[:, q2 * 512:q2 * 512 + w],
                        qk_sb[r0:r0 + 64, 2 + hp, mc * P:(mc + 1) * P],
                        qk_sb[r0:r0 + 64, hp, l0 + q2 * 512:l0 + q2 * 512 + w],
                        start=True, stop=True,
                    )
                clock["pe"] += qn * CYC
                if grp > 1:
                    if mc % grp == grp - 1:
                        E2 = att.tile([P, grp, qn], f16, tag="E", bufs=8,
                                      name=f"E_{h}_{l0}_{mc}")
                        nc.scalar.activation(E2[:], Sgrp[0][:], Exp, scale=0.125)
                        clock["act"] += grp * qn * 0.8333 + 185
                        for i in range(grp):
                            Es[mc - grp + 1 + i] = E2[:, i, :]
                else:
                    E = att.tile([P, qn], f16, tag="E", bufs=8,
                                 name=f"E_{h}_{l0}_{mc}")
                    nc.scalar.activation(E[:], S[:], Exp, scale=0.125)
                    clock["act"] += qn * 0.8333 + 185
                    Es[mc] = E
                for fn in (pending[mc] if mc < len(pending) else ()):
                    fn()
                if mc >= AV_LAG:
                    do_av(mc - AV_LAG)
                drain_fills(bi, mc, blk_pe0, blk_act0)
            for fns in pending[16:]:
                for fn in fns:
                    fn()
            del pending[:]

            def norm_and_tr():
                rsb = att.tile([P, 16], f32, tag="r", bufs=2, name=f"r_{h}_{l0}")
                nc.vector.reciprocal_approx_fast(out=rsb[:, 0:nqc], in_=zt[:, :, 64])
                trs = tr_plan.pop(bi, ())
                # per-chunk normalize when this block's chunks gate its own
                # transposes (shortens the zt -> zn -> out critical path)
                percnk = [x for x in trs if t0 <= x[1] < t0 + nqc] if bi >= 7 else []
                if percnk:
                    done = set()
                    for hp2, t, mode in percnk:
                        nc.vector.tensor_tensor(
                            ztm_sb[:, t, hp, r0:r0 + 64], zt[:, t - t0, 0:64],
                            rsb[:, t - t0, None].to_broadcast((P, 64)), Mult,
                        )
                        done.add(t)
                        tr_emit(hp2, t, mode)
                    rest = [c for c in range(nqc) if t0 + c not in done]
                    if rest:
                        for c in rest:
                            nc.vector.tensor_tensor(
                                ztm_sb[:, t0 + c, hp, r0:r0 + 64],
                                zt[:, c, 0:64],
                                rsb[:, c, None].to_broadcast((P, 64)), Mult,
                            )
                else:
                    nc.vector.tensor_tensor(
                        ztm_sb[:, t0:t0 + nqc, hp, r0:r0 + 64], zt[:, :, 0:64],
                        rsb[:, 0:nqc, None].to_broadcast((P, nqc, 64)), Mult,
                    )
                    for hp2, t, mode in trs:
                        tr_emit(hp2, t, mode)

            pending.append([lambda: do_av(12), lambda: do_av(13)])
            pending.append([lambda: do_av(14)])
            pending.append([lambda: do_av(15)])
            pending.append([norm_and_tr])

        # transposes issued after block bi's norm: (pair, chunk, dual)
        tr_plan = {
            3: [(0, t, "dma") for t in range(8, 16)],
            5: [(None, t, "dual") for t in range(8)],
            7: [(1, t, "pe") for t in range(8, 14)],
            8: [(1, t, "pe") for t in range(14, 16)],
        }

        # ---- fill schedule ----
        # k pair0 chunks 1..15 JIT for B0 (prologue made chunk 0);
        # 256-wide pieces halve the psum-slot churn and DVE evictions
        add_fill((0, 0), lambda: qk_chunk(2, 128))
        for c in (2, 6, 10):
            add_fill((0, max(0, c - 2)),
                     lambda c=c: qk_chunk(2, 128 * c, w=512), cost=1707)
        add_fill((0, 12), lambda: qk_chunk(2, 128 * 14, w=256), cost=854)
        # v heads JIT before their first blocks (B0, B1, B4, B5);
        # paired key-chunks per fill
        for h, bi in ((0, 0), (1, 1), (2, 4), (3, 5)):
            nv = 4 if h == 0 else 2      # B0 is PE-bound: wider is better
            for m in range(0, 16, nv):
                add_fill((bi, min(m + 1, 15)),
                         lambda m=m, h=h, nv=nv: v_chunk(m, h, nv),
                         cost=213 * nv)
        # q pair0 second half (for B2 h0 1024:2048); xT cols land mid-B0
        for i in range(4):
            add_fill((1, 4 + 2 * i),
                     lambda c0=1024 + 256 * i: qk_chunk(0, c0, w=256),
                     ready=(0, 8 + 2 * i), cost=854)
        # q pair1 first half (for B4 h2 / B5 h3 0:1024)
        for i in range(4):
            add_fill((2, 4 + 2 * i), lambda c0=256 * i: qk_chunk(1, c0, w=256),
                     ready=(0, 8 + 2 * i), cost=854)
        # q pair1 second half (for B6 h2 1024:2048)
        for i in range(4):
            add_fill((5, 4 + 2 * i),
                     lambda c0=1024 + 256 * i: qk_chunk(1, c0, w=256),
                     ready=(5, 0), cost=854)
        # k pair1 chunks: 0,1 before B4; rest JIT inside B4 (512-wide)
        add_fill((3, 6), lambda: qk_chunk(3, 0))
        add_fill((3, 8), lambda: qk_chunk(3, 128))
        for c in range(2, 16, 2):
            add_fill((4, max(0, c - 2)),
                     lambda c=c: qk_chunk(3, 128 * c, w=256), cost=854)
        # out t0..7 (zn chunks 0:8 dual-transposed from B6 slot ~5);
        # spread across the ACT-bound blocks B6..B7
        for i in range(16):
            t, oc = i // 2, i % 2
            idx = 6 + (41 * i) // 16
            dl = (6 + idx // 16, idx % 16)
            add_fill(dl, lambda t=t, oc=oc: out_half(t, oc), ready=dl)
        # out t8..13 (zn pair0 8:16 after B3 norm; pair1 8:14 PE-transposed
        # from B8 slot ~5): forced in B8, evictions alternate ACT / DVE
        for i in range(12):
            t, oc = 8 + i // 2, i % 2
            dl = (8, 4 + (11 * i) // 12)
            ev = "act" if i % 2 else "dve"
            add_fill(dl, lambda t=t, oc=oc, ev=ev: out_half(t, oc, ev),
                     ready=dl)
        # ---- block sequence ----
        attn_block(0, 0, 0, 1024)
        attn_block(1, 1, 0, 1024)
        attn_block(2, 0, 1024, 1024)
        attn_block(3, 1, 1024, 1024)
        attn_block(4, 2, 0, 1024)
        attn_block(5, 3, 0, 1024)
        attn_block(6, 2, 1024, 1024)
        attn_block(7, 3, 1024, 768)
        attn_block(8, 3, 1792, 256)

        # tail: flush deferred AVs + norm + transposes, then out t14, t15
        # (per-oc pieces, adds alternating DVE / GpSimd, per-piece DMAs)
        for fns in pending:
            for fn in fns:
                fn()
        del pending[:]
        assert not fillq, f"unemitted fills: {len(fillq)}"
        for t in (14, 15):
            ot = att.tile([P, D], f16, tag=f"tl{t % 2}", bufs=1, name=f"otl_{t}")
            for oc in range(2):
                po = psum.tile([P, 512], f32, tag="S", bufs=3, name=f"pot_{t}_{oc}")
                for dc in range(2):
                    nc.tensor.matmul(
                        po[:],
                        zn_sb[:, dc, t * P:(t + 1) * P],
                        wo_sb[:, dc, oc * 512:(oc + 1) * 512],
                        start=(dc == 0), stop=(dc == 1),
                    )
                if oc == 0:
                    nc.scalar.copy(ot[:, 0:512], po[:])
                else:
                    nc.vector.tensor_copy(ot[:, 512:1024], po[:])
            nc.sync.dma_start(out_ap[:, t, :], ot[:])

    nc.compile()
    return nc


def _get_compiled():
    global _COMPILED
    if _COMPILED is None:
        _COMPILED = _build()
    return _COMPILED


def _shard_inputs(x, W_in, W_out):
    in_maps = []
    xTs = [x[:, b, :].T.astype(np.float16) for b in range(B)]
    for c in range(NCORES):
        b = c // 4
        lo = (c % 4) * J
        Wq = W_in[lo:lo + J]
        Wk = W_in[D + lo:D + lo + J]
        Wv = W_in[2 * D + lo:2 * D + lo + J]
        in_maps.append({
            "xT": xTs[b],
            "wqkT": np.ascontiguousarray(
                np.concatenate([Wq, Wk], 0).T.reshape(KC, P, 4, P)
                .transpose(2, 1, 0, 3).reshape(2 * J, KC * P)
            ).astype(np.float16),
            "wvT": Wv.T.astype(np.float16),
            "woT": np.ascontiguousarray(W_out[:, lo:lo + J].T).astype(np.float16),
            "ident": np.eye(P, dtype=np.float16),
        })
    return in_maps


def _reference_numpy(q, mask, W_in, b_in, W_out, b_out, num_heads):
    l, b, d = q.shape
    hd = d // num_heads
    qkv = q.reshape(l * b, d) @ W_in.T + b_in
    qkv = qkv.reshape(l, b, 3 * d)
    qh, kh, vh = np.split(qkv, 3, axis=-1)

    def to_heads(t):
        return t.reshape(l, b * num_heads, hd).transpose(1, 0, 2)

    qh, kh, vh = to_heads(qh), to_heads(kh), to_heads(vh)
    qh = qh / np.sqrt(np.float32(hd))
    scores = np.einsum("nld,nmd->nlm", qh, kh) + mask
    scores -= scores.max(axis=-1, keepdims=True)
    e = np.exp(scores)
    attn = e / e.sum(axis=-1, keepdims=True)
    z = np.einsum("nlm,nmd->nld", attn, vh)
    z = z.transpose(1, 0, 2).reshape(l * b, d)
    z = z @ W_out.T + b_out
    return z.reshape(l, b, d).astype(np.float32)


def kernel(q, k, v, mask, W_in, b_in, W_out, b_out, num_heads):
    num_heads = int(num_heads)
    q = np.asarray(q, dtype=np.float32)
    W_in = np.asarray(W_in, dtype=np.float32)
    W_out = np.asarray(W_out, dtype=np.float32)
    b_in = np.asarray(b_in, dtype=np.float32)
    b_out = np.asarray(b_out, dtype=np.float32)
    mask = np.asarray(mask, dtype=np.float32)

    if (
        num_heads != H
        or q.shape != (L, B, D)
        or W_in.shape != (3 * D, D)
        or W_out.shape != (D, D)
        or np.any(mask)
        or np.any(b_in)
    ):
        return _reference_numpy(q, mask, W_in, b_in, W_out, b_out, num_heads)

    from concourse import bass_utils

    nc = _get_compiled()
    in_maps = _shard_inputs(q, W_in, W_out)
    res = bass_utils.run_bass_kernel_spmd(
        nc, in_maps, core_ids=list(range(NCORES))
    )

    out = np.zeros((L, B, D), dtype=np.float32)
    for c in range(NCORES):
        out[:, c // 4, :] += res.results[c]["out_p"].astype(np.float32)
    out += b_out
    return out
